# revision 5
# baseline (speedup 1.0000x reference)
"""Differential-Transformer attention (DiffAttn) Trainium2 Bass kernel, v3.

Sharding: 8 cores = 2 (batch) x 4 (head-group tensor parallel).
Core c = 4*b + t handles batch b, query heads 4t..4t+3, kv head t, and the
effective v weights its query heads need.  o_proj is row-parallel: each core
returns a partial [L, HID] product; the host sums the 4 partials per batch.

Design notes (v3):
- Host folds: wv_eff = Wv_lo - lambda*Wv_hi (differential combine collapses
  into the V projection -> one 128-wide V head per core), and
  rms_weight*(1-lambda_init) folded into o_proj rows.
- All matmul operands stay f32/f32r: a single bf16 tensor anywhere in the
  main chain measurably costs ~1.4e-2 mean rel err (the 2e-2 budget does
  not tolerate 8-bit-mantissa operands).  Only the causal masks (exact 0/1)
  and the output partials ship as bf16.
- Single-pass pipeline over q-slices j=0..3 (ascending): projections(j) ->
  attention(j) -> o_proj(j).  Attention j needs only k/v blocks <= j, so
  projections(j+1) fill PE while attention(j) waits on ACT's exp, and
  o_proj(j) fills PE during the exp/norm-bound attention stretches.
- No staging copies: DMA lands directly in f32r tiles (dram tensors are
  declared f32r; bits are f32).  hs is streamed chunk-granular through a
  17-slot pool; weights are resident.
- Softmax-denominator / RMS-norm fold: den cancels through the rsqrt (eps
  negligible), comb = exp(-0.5*ln(mean_d u^2)); the partition-dim sum of
  squares runs on GpSimd (partition_all_reduce), sparing PE and a PSUM
  bank.  Exp/Ln/Copy all live in one ACT LUT set -> a single table load.
- Causal structure: only kb blocks <= diagonal are computed; for j >= 1
  the three diagonal sub-blocks shrink their score/AV matmuls to the
  valid q-range and the mask multiplies shrink to one 128-wide block
  (j=0 keeps full width: PSUM/SBUF pool slots hold garbage on first use,
  and full-width exp+mask launders it).
- Engine split: PE matmuls; DVE RoPE muls + psum evacuation; ACT exp/ln +
  psum evacuation; GpSimd mask muls, u^2, partition reduce, RoPE add/sub.
- Score pairs: two [128,512] score matmuls share a 2-bank psum tile, one
  Exp covers [128,1024].  Diagonal pairs are emitted FIRST so their exp +
  mask multiplies run while PE works through the earlier pairs' AV.
- PSUM exactly 8 banks: proj 2x[128,512], score pairs 2x[128,1024],
  AV/o_proj shared 2x[128,512].
"""

import os
import sys

import numpy as np

for _p in ("/opt/trn_rl_repo",):
    if _p not in sys.path and os.path.isdir(_p):
        sys.path.insert(0, _p)

B = 2
L = 2048
HID = 2048
D = 128
H = 16
NH = 4            # query heads per core
CT = HID // 128   # contraction tiles for the projections
EPS = 1e-6
LAMBDA_INIT = 0.2

_CACHE = {}


def _build(length=L):
    from concourse import bacc
    import concourse.mybir as mybir
    import concourse.tile as tile

    f32 = mybir.dt.float32
    f32r = mybir.dt.float32r
    bf16 = mybir.dt.bfloat16
    Act = mybir.ActivationFunctionType

    # Exp, Ln and Copy all live in the 'natural_log_exp_and_others' LUT set,
    # but the table-load pass assigns each activation the first set
    # containing its function, which alternates sets and inserts a ~1.3us
    # table reload per switch.  Restrict the pass's view to that one set
    # (keeping list positions so act_func_set_id indices stay valid) ->
    # exactly one load total.
    _orig_tables = bacc.get_activation_tables

    def _only_ln_exp(arch):
        t = _orig_tables(arch)
        keep = "natural_log_exp_and_others"
        if keep not in t:
            return t
        return {name: (s if name == keep else set()) for name, s in t.items()}

    bacc.get_activation_tables = _only_ln_exp
    try:
        return _build_inner(length, bacc, mybir, tile, f32, f32r, bf16, Act)
    finally:
        bacc.get_activation_tables = _orig_tables


def _build_inner(length, bacc, mybir, tile, f32, f32r, bf16, Act):
    from concourse import bass_isa

    NJ = length // 512    # q-slices
    NLB = length // 128   # l/k blocks
    WQKV = NH * D + 2 * D  # q heads | k | v_eff columns

    f16 = mybir.dt.float16

    nc = bacc.Bacc()
    # hs and the weights ship as fp16 (11 sig bits); the weights carry a
    # x64 scale so their mass sits in fp16's well-conditioned range.  The
    # scale cancels exactly: q/k scales fold into the exp scale, the veff
    # scale cancels through the RMS rsqrt, and the o_proj scale is divided
    # out on the host.
    hsT = nc.dram_tensor("hsT", [HID, length], f16, kind="ExternalInput")
    cosT = nc.dram_tensor("cosT", [D, length], f32, kind="ExternalInput")
    sinT = nc.dram_tensor("sinT", [D, length], f32, kind="ExternalInput")
    wqkv = nc.dram_tensor("wqkv", [HID, WQKV], f16, kind="ExternalInput")
    wo = nc.dram_tensor("wo", [NH * D, HID], f16, kind="ExternalInput")
    masks = nc.dram_tensor("masks", [D, 1280], bf16, kind="ExternalInput")
    part = nc.dram_tensor("part", [length, HID], f32, kind="ExternalOutput")

    inv_sqrt_d = 1.0 / np.sqrt(np.float32(D))

    with tile.TileContext(nc) as tc:
        with tc.tile_pool(name="persist", bufs=1) as persist:
            wqkv_t = persist.tile([D, CT, WQKV], f16, tag="wqkv")
            wo_t = persist.tile([D, NH, HID], f16, tag="wo")
            kT = persist.tile([D, length], f16, tag="kT")
            veff = persist.tile([D, NLB, D], f16, tag="veff")
            # causal masks for the 4 diagonal sub-blocks, packed: block d
            # covers q-range [0, 128*(d+1)) at offset MOFF[d]
            mask_t = persist.tile([D, 1280], bf16, tag="mask")
            MOFF = (0, 128, 384, 768)

            wqkv_r = wqkv.rearrange("(c p) m -> p c m", p=D)
            wo_r = wo.rearrange("(h p) m -> p h m", p=D)

            with tc.tile_pool(name="hpool", bufs=24) as hpool, \
                 tc.tile_pool(name="qpool", bufs=2) as qpool, \
                 tc.tile_pool(name="fpool", bufs=2) as fpool, \
                 tc.tile_pool(name="cspool", bufs=2) as cspool, \
                 tc.tile_pool(name="sepool", bufs=4) as sepool, \
                 tc.tile_pool(name="ufpool", bufs=2) as ufpool, \
                 tc.tile_pool(name="npool", bufs=2) as npool, \
                 tc.tile_pool(name="sqpool", bufs=1) as sqpool, \
                 tc.tile_pool(name="btmp", bufs=2) as btmp, \
                 tc.tile_pool(name="obpool", bufs=4) as obpool, \
                 tc.tile_pool(name="projp", bufs=2, space="PSUM") as projp, \
                 tc.tile_pool(name="pairp", bufs=2, space="PSUM") as pairp, \
                 tc.tile_pool(name="outp", bufs=2, space="PSUM") as outp:

                # ---- startup DMAs -------------------------------------
                # Interleave weight chunk + j=0 hs chunk per c so the first
                # projection groups ride the DMA arrivals; cos/sin(0) early
                # (RoPE j=0), then hs(1), wo, hs(2), hs(3).
                hs_chunks = {}

                def load_hs(c, j):
                    t = hpool.tile([D, 512], f16, tag="hs")
                    nc.sync.dma_start(
                        out=t, in_=hsT[128 * c:128 * (c + 1),
                                       512 * j:512 * (j + 1)])
                    hs_chunks[(c, j)] = t

                cs_t = {}

                def load_cs(j):
                    cos_s = cspool.tile([D, 512], f32, tag="cos")
                    sin_s = cspool.tile([D, 512], f32, tag="sin")
                    nc.sync.dma_start(out=cos_s,
                                      in_=cosT[:, 512 * j:512 * (j + 1)])
                    nc.sync.dma_start(out=sin_s,
                                      in_=sinT[:, 512 * j:512 * (j + 1)])
                    cs_t[j] = (cos_s, sin_s)

                # wqkv columns are host-ordered [k | q0 | v | q1 | q2 | q3]
                # so the first 512 columns feed the four startup projection
                # groups; q2/q3 columns follow in one transfer while RoPE
                # drains.  Everything is ordered by first-need on one queue.
                nc.sync.dma_start(out=wqkv_t[:, 0, 0:512],
                                  in_=wqkv_r[:, 0, 0:512])
                load_hs(0, 0)
                nc.sync.dma_start(out=wqkv_t[:, 1:4, 0:512],
                                  in_=wqkv_r[:, 1:4, 0:512])
                for c in range(1, 4):
                    load_hs(c, 0)
                for g in range(1, CT // 4):
                    nc.sync.dma_start(
                        out=wqkv_t[:, 4 * g:4 * (g + 1), 0:512],
                        in_=wqkv_r[:, 4 * g:4 * (g + 1), 0:512])
                    for c in range(4 * g, 4 * (g + 1)):
                        load_hs(c, 0)
                load_cs(0)
                nc.sync.dma_start(out=mask_t, in_=masks[:, :])
                nc.sync.dma_start(out=wqkv_t[:, :, 512:768],
                                  in_=wqkv_r[:, :, 512:768])
                load_cs(1)
                # hs(1) chunks interleaved with wo osl-quarters: proj(1)
                # rides the chunk arrivals while o_proj(0) (osl-major) can
                # start as soon as its quarter lands.
                for c in range(CT):
                    load_hs(c, 1)
                    if c % 4 == 3:
                        q4 = c // 4
                        nc.sync.dma_start(
                            out=wo_t[:, :, 512 * q4:512 * (q4 + 1)],
                            in_=wo_r[:, :, 512 * q4:512 * (q4 + 1)])

                for j in range(NJ):
                    nkb = 4 * j + 4
                    npair = nkb // 2
                    sl = slice(512 * j, 512 * (j + 1))
                    cos_s, sin_s = cs_t.pop(j)

                    # ---- projections + RoPE: q heads + k in [D, l] ----
                    # Group order k, q0, v, q1..q3: attention h=0 unblocks
                    # (kT, qT0, veff) as early as possible.
                    qTj = qpool.tile([D, NH, 512], f16, tag="qT")
                    # wqkv column layout: [k | q0 | v | q1 | q2 | q3]
                    QCOL = (128, 384, 512, 640)
                    KCOL, VCOL = 0, 256

                    def proj_rope(db, pool):
                        # db == NH means the k head
                        ps = pool.tile([D, 512], f32, tag=pool._ptag)
                        col = KCOL if db == NH else QCOL[db]
                        for c in range(CT):
                            nc.tensor.matmul(ps, wqkv_t[:, c, col:col + 128],
                                             hs_chunks[(c, j)],
                                             start=(c == 0),
                                             stop=(c == CT - 1))
                        # RoPE: rotate_half swaps partition halves; sin's
                        # halves are identical, so rot(q)*sin lands via
                        # partition-swapped reads of the PSUM tile.  dst is
                        # first ps*cos (DVE), then the rotated sin term is
                        # applied in place half-wise on GpSimd.
                        t2 = btmp.tile([D, 512], f32, tag="t2")
                        if db < NH:
                            dst = qTj[:, db, :]
                            dst_lo = qTj[0:64, db, :]
                            dst_hi = qTj[64:128, db, :]
                        else:
                            dst = kT[:, sl]
                            dst_lo = kT[0:64, sl]
                            dst_hi = kT[64:128, sl]
                        nc.vector.tensor_mul(dst, ps, cos_s)
                        nc.vector.tensor_mul(t2[0:64, :], ps[64:128, :],
                                             sin_s[0:64, :])
                        nc.vector.tensor_mul(t2[64:128, :], ps[0:64, :],
                                             sin_s[64:128, :])
                        nc.gpsimd.tensor_sub(dst_lo, dst_lo, t2[0:64, :])
                        nc.gpsimd.tensor_add(dst_hi, dst_hi, t2[64:128, :])

                    def proj_v(pool):
                        # effective V projection straight in [l, d] layout:
                        # hs chunk stationary, wv_eff moving — fp16 matmuls
                        # have no <256-moving penalty, so the 128-wide
                        # moving dim runs at full rate.
                        psv = pool.tile([D, 4, 128], f32, tag=pool._ptag)
                        for i in range(4):
                            for c in range(CT):
                                nc.tensor.matmul(
                                    psv[:, i, :],
                                    hs_chunks[(c, j)][:,
                                                      128 * i:128 * (i + 1)],
                                    wqkv_t[:, c, VCOL:VCOL + 128],
                                    start=(c == 0), stop=(c == CT - 1))
                        nc.scalar.copy(out=veff[:, 4 * j:4 * j + 4, :],
                                       in_=psv)

                    projp._ptag = "proj"
                    outp._ptag = "av"
                    if j == 0:
                        # outp is idle until the first attention ops: borrow
                        # its banks so four projection groups ride the
                        # startup DMA arrivals concurrently.
                        proj_rope(NH, projp)
                        proj_rope(0, projp)
                        proj_v(outp)
                        proj_rope(1, outp)
                        proj_rope(2, projp)
                        proj_rope(3, projp)
                    else:
                        proj_rope(NH, projp)
                        proj_rope(0, projp)
                        proj_v(projp)
                        for db in (1, 2, 3):
                            proj_rope(db, projp)

                    # release this slice's hs chunks (tile pool frees them
                    # once the last reader -- the projections above -- ran).
                    for c in range(CT):
                        hs_chunks.pop((c, j))

                    # ---- attention ------------------------------------
                    shrink = j > 0
                    finalTj = fpool.tile([D, NH, 512], f16, tag="finalT")

                    for h in range(NH):
                        pso = outp.tile([D, 512], f32, tag="av")
                        se_l = {}

                        def score_pair(t):
                            pss = pairp.tile([D, 2, 512], f32, tag="s")
                            for i in (0, 1):
                                kb = 2 * t + i
                                d = kb - 4 * j  # diag sub-block index
                                lo = 128 * d if shrink and 1 <= d <= 3 \
                                    else 0
                                nc.tensor.matmul(
                                    pss[:, i, lo:512],
                                    kT[:, 128 * kb:128 * (kb + 1)],
                                    qTj[:, h, lo:512],
                                    start=True, stop=True)
                            se = sepool.tile([D, 2, 512], f16, tag="se")
                            # scores carry the (64 q) x (64 k) weight scale
                            nc.scalar.activation(se, pss, Act.Exp,
                                                 scale=float(inv_sqrt_d
                                                             / 4096.0))
                            # causal masks on the diagonal sub-blocks
                            for i in (0, 1):
                                kb = 2 * t + i
                                d = kb - 4 * j
                                if 0 <= d <= 3:
                                    # all-2-byte SBUF operands -> DVE 2x
                                    lo = 128 * d if shrink else 0
                                    nc.vector.tensor_mul(
                                        se[:, i, lo:128 * (d + 1)],
                                        se[:, i, lo:128 * (d + 1)],
                                        mask_t[:, MOFF[d] + lo:
                                               MOFF[d] + 128 * (d + 1)])
                            se_l[t] = se

                        # diagonal pairs first: their exp + mask muls run
                        # while PE chews the earlier pairs' AV.
                        score_pair(2 * j)
                        score_pair(2 * j + 1)
                        if npair > 2:
                            score_pair(0)
                        for t in range(npair):
                            if t + 1 <= npair - 3:
                                score_pair(t + 1)
                            se = se_l.pop(t)
                            for i in (0, 1):
                                kb = 2 * t + i
                                d = kb - 4 * j
                                lo = 128 * d if shrink and 1 <= d <= 3 \
                                    else 0
                                nc.tensor.matmul(pso[:, lo:512],
                                                 veff[:, kb, :],
                                                 se[:, i, lo:512],
                                                 start=(kb == 0),
                                                 stop=(kb == nkb - 1))

                        # Normalization: softmax 1/den and the RMS rsqrt
                        # fold into comb = rsqrt(mean_d(u^2)) (den cancels,
                        # eps negligible); partition-dim sum on GpSimd.
                        u_f = ufpool.tile([D, 512], f32, tag="uf")
                        nc.vector.tensor_copy(u_f, pso)
                        sq = sqpool.tile([D, 512], f32, tag="sq")
                        nc.vector.tensor_mul(sq, u_f, u_f)
                        nc.gpsimd.partition_all_reduce(
                            sq, sq, D, bass_isa.ReduceOp.add)
                        lnr = npool.tile([D, 512], f32, tag="lc")
                        nc.scalar.activation(lnr, sq, Act.Ln,
                                             scale=1.0 / 128.0)
                        comb = npool.tile([D, 512], f32, tag="lc")
                        nc.scalar.activation(comb, lnr, Act.Exp, scale=-0.5)
                        nc.vector.tensor_mul(finalTj[:, h, :], u_f, comb)

                    # ---- o_proj for this q-slice (osl-major) ----------
                    nt = 0
                    for osl in range(4):
                        osl_s = slice(512 * osl, 512 * (osl + 1))
                        for lb in range(4 * j, 4 * j + 4):
                            lsl = slice(128 * lb, 128 * (lb + 1))
                            ql = slice(128 * (lb - 4 * j),
                                       128 * (lb - 4 * j + 1))
                            ps = outp.tile([D, 512], f32, tag="av")
                            for h in range(NH):
                                nc.tensor.matmul(ps, finalTj[:, h, ql],
                                                 wo_t[:, h, osl_s],
                                                 start=(h == 0),
                                                 stop=(h == NH - 1))
                            ob = obpool.tile([D, 512], f32, tag="ob")
                            if j < 2 and nt % 4 == 3:
                                nc.scalar.copy(out=ob, in_=ps)
                            else:
                                nc.vector.tensor_copy(ob, ps)
                            nc.sync.dma_start(out=part[lsl, osl_s], in_=ob)
                            nt += 1

                    # prefetch hs / cos / sin for slice j+2 — emitted after
                    # this slice's out-DMAs so the prefetch stream (which
                    # throttles on hpool slots) can't head-of-line block
                    # them on the SP queue.
                    if j + 2 < NJ:
                        for c in range(CT):
                            load_hs(c, j + 2)
                        load_cs(j + 2)

    nc.finalize()
    return nc


def _causal_masks():
    # packed [D, 1280]: diagonal sub-block d covers q-range [0, 128*(d+1))
    # at offset {0, 128, 384, 768}[d]
    m = np.zeros((D, 1280), np.float32)
    off = (0, 128, 384, 768)
    for r in range(4):
        for p in range(D):
            q0 = 128 * r + p
            m[p, off[r] + q0: off[r] + 128 * (r + 1)] = 1.0
    return m


def kernel(hidden_states, cos, sin, Wq, Wk, Wv, Wo,
           lambda_q1, lambda_k1, lambda_q2, lambda_k2, rms_weight):
    import ml_dtypes
    from concourse.bass_utils import run_bass_kernel_spmd

    bf = ml_dtypes.bfloat16
    length = hidden_states.shape[1]
    if length not in _CACHE:
        _CACHE[length] = _build(length)
    nc = _CACHE[length]

    hidden_states = np.asarray(hidden_states, np.float32)
    cos = np.asarray(cos, np.float32)
    sin = np.asarray(sin, np.float32)

    lam_full = np.float32(
        np.exp(np.float32(np.dot(np.asarray(lambda_q1, np.float32),
                                 np.asarray(lambda_k1, np.float32)))
               + np.float32(np.dot(np.asarray(lambda_q2, np.float32),
                                   np.asarray(lambda_k2, np.float32))))
        + np.float32(LAMBDA_INIT))
    rms_scale = (np.asarray(rms_weight, np.float32)
                 * np.float32(1.0 - LAMBDA_INIT))          # [D]
    masks = _causal_masks().astype(bf)

    Wq = np.asarray(Wq, np.float32)
    Wk = np.asarray(Wk, np.float32)
    Wv = np.asarray(Wv, np.float32)
    Wo = np.asarray(Wo, np.float32)

    in_maps = []
    for b in range(B):
        hsT_b = np.ascontiguousarray(hidden_states[b].T).astype(np.float16)
        cosT_b = np.ascontiguousarray(cos[b].T)
        sinT_b = np.ascontiguousarray(sin[b].T)
        for t in range(4):
            vlo, vhi = t // 2, t // 2 + 2
            wv_eff = (Wv[:, 128 * vlo:128 * (vlo + 1)]
                      - lam_full * Wv[:, 128 * vhi:128 * (vhi + 1)])
            wo_eff = (Wo[512 * t:512 * (t + 1), :]
                      * np.tile(rms_scale, NH)[:, None])
            # column order [k | q0 | v_eff | q1 | q2 | q3] (startup streams
            # the first 512 columns first)
            Wq_t = Wq[:, 512 * t:512 * (t + 1)]
            wqkv = np.concatenate(
                [Wk[:, 128 * t:128 * (t + 1)], Wq_t[:, 0:128], wv_eff,
                 Wq_t[:, 128:256], Wq_t[:, 256:384], Wq_t[:, 384:512]],
                axis=1)
            in_maps.append({
                "hsT": hsT_b,
                "cosT": cosT_b,
                "sinT": sinT_b,
                "wqkv": np.ascontiguousarray(wqkv * np.float32(64.0)
                                             ).astype(np.float16),
                "wo": np.ascontiguousarray(wo_eff * np.float32(64.0)
                                           ).astype(np.float16),
                "masks": masks,
            })

    trace = bool(os.environ.get("DIFFATTN_TRACE"))
    res = run_bass_kernel_spmd(nc, in_maps, list(range(8)), trace=trace)
    kernel.last_results = res

    out = np.empty((B, length, HID), np.float32)
    for b in range(B):
        acc = res.results[4 * b]["part"].astype(np.float32)
        for t in range(1, 4):
            acc += res.results[4 * b + t]["part"].astype(np.float32)
        out[b] = acc * np.float32(1.0 / 64.0)   # undo the wo x64 scale
    return out
